# revision 1
# baseline (speedup 1.0000x reference)
"""GCN encoder Bass kernel for 8 TRN2 NeuronCores.

Strategy: nodes are degree-sorted/snake-sharded across the 8 cores (6250 real
+ 22 pad slots each). Each layer: PE transforms the local shard (stationary =
feature-major input tile, moving = weight), ACT scales by dinv + casts to bf16
node-major, remote_dma_broadcast allgathers all shards into every core's SBUF
token buffer, SWDGE dma_gather (two int16 base-offset views) pulls per-edge
source rows feature-major, DVE grouped-reduces them into the aggregation
buffer, then dinv-scale + global BN stats (bn_stats/bn_aggr + tiny stats
broadcast) + fused relu-affine on ACT.
"""
import numpy as np
import ml_dtypes
from contextlib import ExitStack

import concourse.bass as bass
import concourse.bacc as bacc
import concourse.mybir as mybir

N, E, FIN, H, OUT = 50000, 800000, 128, 128, 64
NCORES = 8
SHARD = 6272
REAL = 6250
NT = SHARD // 128          # 49
NSLOT = NCORES * SHARD     # 50176
YN_RANKS = 394             # rank 0 zeros(A) | 392 data | rank 393 zeros(B)
YN_ELEMS = YN_RANKS * 128  # 50432 bf16 per partition
A_BASE = 128
B_SHIFT = 17536
A_MAX_V = 32639
B_MIN_V = 17536
ZB_BASE = 32640
B_VIEW_RANK = 138
GCAP_COLS = 10240
BN_EPS = 1e-5
bf16 = ml_dtypes.bfloat16
f32 = mybir.dt.float32
bfl = mybir.dt.bfloat16
AF = mybir.ActivationFunctionType
AL = mybir.AluOpType


def preprocess(edge_index):
    src = edge_index[0].astype(np.int64)
    dst = edge_index[1].astype(np.int64)
    deg_in = np.bincount(dst, minlength=N)
    deg = (deg_in + 1).astype(np.float64)
    dinv = (1.0 / np.sqrt(deg)).astype(np.float32)

    src_all = np.concatenate([src, np.arange(N)])
    dst_all = np.concatenate([dst, np.arange(N)])
    tot = deg_in + 1

    def assign(order):
        rank = np.arange(N)
        rnd = rank // NCORES
        pos = rank % NCORES
        core_of_rank = np.where(rnd % 2 == 0, pos, NCORES - 1 - pos)
        slot_global = np.empty(N, np.int64)
        node_of_slot = np.full(NSLOT, -1, np.int64)
        for c in range(NCORES):
            nodes_c = order[core_of_rank == c]
            slot_global[nodes_c] = c * SHARD + np.arange(len(nodes_c))
            node_of_slot[c * SHARD + np.arange(len(nodes_c))] = nodes_c
        return slot_global, node_of_slot

    def classify(slot_global):
        sslot = slot_global[src_all]
        na = np.zeros(N, np.int64)
        nb = np.zeros(N, np.int64)
        nm = np.zeros(N, np.int64)
        isa = sslot < B_MIN_V
        isb = sslot > A_MAX_V
        ism = ~isa & ~isb
        np.add.at(na, dst_all[isa], 1)
        np.add.at(nb, dst_all[isb], 1)
        np.add.at(nm, dst_all[ism], 1)
        return na, nb, nm

    order0 = np.argsort(-tot, kind="stable")
    rank = np.arange(N)
    rnd = rank // NCORES
    pos = rank % NCORES
    core_of_rank = np.where(rnd % 2 == 0, pos, NCORES - 1 - pos)
    slot_global, node_of_slot = assign(order0)
    for _ in range(2):
        na, nb, nm = classify(slot_global)
        sg2 = np.empty(N, np.int64)
        ns2 = np.full(NSLOT, -1, np.int64)
        for c in range(NCORES):
            nodes_c = order0[core_of_rank == c]
            k = np.lexsort((-(na[nodes_c] - nb[nodes_c]), -(tot[nodes_c] // 3)))
            nodes_c = nodes_c[k]
            sg2[nodes_c] = c * SHARD + np.arange(len(nodes_c))
            ns2[c * SHARD + np.arange(len(nodes_c))] = nodes_c
        slot_global, node_of_slot = sg2, ns2

    sslot = slot_global[src_all]
    dslot = slot_global[dst_all]
    order_e = np.argsort(dslot, kind="stable")
    sslot_s = sslot[order_e]
    counts = np.bincount(dslot[order_e], minlength=NSLOT)
    starts = np.concatenate([[0], np.cumsum(counts)])

    SA = np.zeros(NT, np.int64)
    SB = np.zeros(NT, np.int64)
    a_lists = [None] * NSLOT
    b_lists = [None] * NSLOT
    for t in range(NT):
        info = []
        for c in range(NCORES):
            for p in range(128):
                s = c * SHARD + t * 128 + p
                nb_ = sslot_s[starts[s]:starts[s + 1]]
                a = nb_[nb_ < B_MIN_V]
                b = nb_[nb_ > A_MAX_V]
                f = nb_[(nb_ >= B_MIN_V) & (nb_ <= A_MAX_V)]
                info.append((s, a, b, f))
        amax = max(len(a) for _, a, _, _ in info)
        afmax = max(len(a) + len(f) for _, a, _, f in info)
        best = None
        for sa_c in range(amax, afmax + 1):
            sb_need = max(len(b) + max(0, len(a) + len(f) - sa_c)
                          for _, a, b, f in info)
            sa_e = (sa_c + 1) // 2 * 2
            sb_e = (sb_need + 1) // 2 * 2
            if best is None or sa_e + sb_e < best[0] + best[1]:
                best = (sa_e, sb_e, sa_c)
        sa_e, sb_e, sa_c = best
        SA[t], SB[t] = max(sa_e, 2), max(sb_e, 2)
        for s, a, b, f in info:
            take = min(max(0, sa_c - len(a)), len(f))
            a_lists[s] = np.concatenate([a, f[:take]])
            b_lists[s] = np.concatenate([b, f[take:]])

    chunks = []
    t0 = 0
    cols = 0
    for t in range(NT):
        tc = 128 * (SA[t] + SB[t])
        if t > t0 and cols + tc > GCAP_COLS:
            chunks.append((t0, t))
            t0, cols = t, 0
        cols += tc
    chunks.append((t0, NT))
    gslot_cols = max(sum(128 * (SA[t] + SB[t]) for t in range(a, b))
                     for a, b in chunks)

    idx_streams = []
    for c in range(NCORES):
        parts = []
        for (ta, tb) in chunks:
            for t in range(ta, tb):
                for p in range(128):
                    s = c * SHARD + t * 128 + p
                    a = a_lists[s] + A_BASE
                    pad = np.full(SA[t] - len(a), p, np.int64)
                    parts.append(np.concatenate([a, pad]))
            for t in range(ta, tb):
                for p in range(128):
                    s = c * SHARD + t * 128 + p
                    b = b_lists[s] - B_SHIFT
                    pad = np.full(SB[t] - len(b), ZB_BASE + p, np.int64)
                    parts.append(np.concatenate([b, pad]))
        stream = np.concatenate(parts)
        assert stream.min() >= 0 and stream.max() <= 32767
        idx_streams.append(stream.astype(np.int16))

    total_cols = len(idx_streams[0])
    idx_dram = np.zeros((NCORES, 128, total_cols // 16), np.int16)
    for c in range(NCORES):
        w = idx_streams[c].reshape(-1, 16).T
        for g in range(8):
            idx_dram[c, 16 * g:16 * (g + 1), :] = w

    dinv_slot = np.zeros(NSLOT, np.float32)
    m = node_of_slot >= 0
    dinv_slot[m] = dinv[node_of_slot[m]]

    return dict(dinv_slot=dinv_slot, node_of_slot=node_of_slot,
                SA=SA, SB=SB, chunks=chunks, gslot_cols=gslot_cols,
                idx_dram=idx_dram, total_cols=total_cols)


class Sem:
    """semaphore + python-side cumulative counter"""
    def __init__(self, nc, name):
        self.h = nc.alloc_semaphore(name)
        self.n = 0

    def inc(self, inst, k):
        inst.then_inc(self.h, k)
        self.n += k
        return self.n


def build_program(pp, layers=4, do_bcast=True, do_gather=True, do_stats=True, debug_dump=False):
    SA, SB, chunks = pp["SA"], pp["SB"], pp["chunks"]
    gslot_cols = pp["gslot_cols"]
    idx_cols = pp["total_cols"] // 16
    nchunks = len(chunks)
    maxtiles = max(tb - ta for ta, tb in chunks)

    nc = bacc.Bacc("TRN2", target_bir_lowering=False, debug=False,
                   num_devices=NCORES)

    # DRAM I/O
    xT_d = nc.dram_tensor("xT", [128, SHARD], f32, kind="ExternalInput")
    idx_d = nc.dram_tensor("idx", [128, idx_cols], mybir.dt.int16,
                           kind="ExternalInput")
    drep_d = nc.dram_tensor("drep", [128, SHARD], bfl, kind="ExternalInput")
    dnode_d = nc.dram_tensor("dnode", [128, NT], f32, kind="ExternalInput")
    wall_d = nc.dram_tensor("wall", [128, 512], f32, kind="ExternalInput")
    gb_d = nc.dram_tensor("gb", [128, 8], f32, kind="ExternalInput")
    out_d = nc.dram_tensor("out", [128, SHARD], f32, kind="ExternalOutput")
    if debug_dump:
        dbg_stage = nc.dram_tensor("dbg_stage", [128, SHARD], bfl,
                                   kind="ExternalOutput")
        dbg_yn = nc.dram_tensor("dbg_yn", [128, YN_ELEMS], bfl,
                                kind="ExternalOutput")
        dbg_g = nc.dram_tensor("dbg_g", [128, pp["gslot_cols"]], bfl,
                               kind="ExternalOutput")

    ctx = ExitStack()
    # SBUF
    yn = ctx.enter_context(nc.sbuf_tensor([128, YN_ELEMS], bfl))
    idx_sb = ctx.enter_context(nc.sbuf_tensor([128, idx_cols], mybir.dt.int16))
    G = [ctx.enter_context(nc.sbuf_tensor(f"G{i}", [128, gslot_cols], bfl))
         for i in range(2)]
    acc = ctx.enter_context(nc.sbuf_tensor([128, SHARD], f32))
    drep = ctx.enter_context(nc.sbuf_tensor([128, SHARD], bfl))
    stage = ctx.enter_context(nc.sbuf_tensor([128, SHARD], bfl))

    wsb = ctx.enter_context(nc.sbuf_tensor([128, 512], f32))
    dnode = ctx.enter_context(nc.sbuf_tensor([128, NT], f32))
    gbv = ctx.enter_context(nc.sbuf_tensor([128, 8], f32))
    accA = ctx.enter_context(nc.sbuf_tensor([128, 128], f32))
    accB = ctx.enter_context(nc.sbuf_tensor([128, 128], f32))
    stats6 = ctx.enter_context(nc.sbuf_tensor([128, 13 * 6], f32))
    mv = ctx.enter_context(nc.sbuf_tensor([128, 8], f32))
    xch_s = ctx.enter_context(nc.sbuf_tensor([128, 2], f32))
    xch_r = ctx.enter_context(nc.sbuf_tensor([128, 16], f32))
    kvec = ctx.enter_context(nc.sbuf_tensor([128, 1], f32))
    bvec = ctx.enter_context(nc.sbuf_tensor([128, 1], f32))
    t0v = ctx.enter_context(nc.sbuf_tensor([128, 1], f32))
    t1v = ctx.enter_context(nc.sbuf_tensor([128, 1], f32))
    t2v = ctx.enter_context(nc.sbuf_tensor([128, 1], f32))
    t2av = ctx.enter_context(nc.sbuf_tensor([128, 1], f32))
    s2v = ctx.enter_context(nc.sbuf_tensor([128, 2], f32))
    # one full 2KB PSUM bank per tile: concurrent PE-write + ACT-read in the
    # same bank is a hardware fault, so never co-locate two tiles in a bank.
    ps_full = [ctx.enter_context(nc.psum_tensor(f"ps{i}", [128, 512], f32))
               for i in range(4)]
    ps = [p[:, 0:128] for p in ps_full]
    ps_dummy = ctx.enter_context(nc.psum_tensor("psd", [128, 512], f32))

    # semaphores
    ld = Sem(nc, "ld"); xs = Sem(nc, "xs"); mm = Sem(nc, "mm")
    ynS = Sem(nc, "ynS"); bn = Sem(nc, "bn"); gd = Sem(nc, "gd")
    gq = Sem(nc, "gq"); rs = Sem(nc, "rs"); ls = Sem(nc, "ls")
    dn = Sem(nc, "dn"); dl = Sem(nc, "dl"); psm = Sem(nc, "psm")
    srs = Sem(nc, "srs"); sls = Sem(nc, "sls"); sqr = Sem(nc, "sqr")
    kb = Sem(nc, "kb"); st = Sem(nc, "st"); sq = Sem(nc, "sq")
    od = Sem(nc, "od"); fv = Sem(nc, "fv"); fa = Sem(nc, "fa")

    # per-chunk A/B column counts and idx column offsets
    chunk_meta = []
    icol = 0
    for (ta, tb) in chunks:
        colsA = int(sum(128 * SA[t] for t in range(ta, tb)))
        colsB = int(sum(128 * SB[t] for t in range(ta, tb)))
        chunk_meta.append((ta, tb, colsA, colsB, icol, icol + colsA // 16))
        icol += (colsA + colsB) // 16
    assert icol == idx_cols

    with nc.Block() as block:

        @block.sync
        def _(sp):
            for d_, s_ in [(idx_sb, idx_d), (drep, drep_d), (dnode, dnode_d),
                           (wsb, wall_d), (gbv, gb_d)]:
                sp.dma_start(d_[:], s_[:]).then_inc(ld.h, 16)
            ld.n = 80
            # layer-0 input loads straight into acc: the aggregation's first
            # write to acc is ordered after every layer-0 transform read.
            sp.dma_start(acc[:], xT_d[:]).then_inc(xs.h, 16)
            xs.n += 16
            if debug_dump:
                sp.wait_ge(kb.h, layers)
                if do_stats:
                    sp.wait_ge(sqr.h, min(layers, 3))
                sp.dma_start(dbg_stage[:], stage[:]).then_inc(od.h, 16)
                od.n += 16
                sp.dma_start(dbg_yn[:], yn[:]).then_inc(od.h, 16)
                od.n += 16
                with nc.allow_non_contiguous_dma(reason="debug dumps"):
                    for j, src_ap in enumerate([xch_r[:], xch_s[:], mv[:],
                                                kvec[:], bvec[:], t0v[:],
                                                t1v[:], s2v[:], stats6[:]]):
                        w = src_ap.shape[1]
                        sp.dma_start(dbg_g.bitcast(f32)[:, 40*j:40*j+w],
                                     src_ap).then_inc(od.h, 16)
                        od.n += 16
            sp.wait_ge(bn.h, layers if (do_stats and layers == 4) else 0)
            if not (do_stats and layers == 4):
                sp.wait_ge(kb.h, layers)
            sp.dma_start(out_d[:], acc[:]).then_inc(od.h, 16)
            od.n += 16
            sp.wait_ge(od.h, od.n)

        @block.tensor
        def _(te):
            te.wait_ge(ld.h, 80)
            for l in range(layers):
                for t in range(NT):
                    i = l * NT + t
                    if l == 0:
                        if t == 0:
                            te.wait_ge(xs.h, 16)
                        lhsT = acc[:, t * 128:(t + 1) * 128]
                    else:
                        if t == 0:
                            te.wait_ge(bn.h, l)
                        lhsT = acc[:, t * 128:(t + 1) * 128]
                    if i >= 4:
                        te.wait_ge(ynS.h, i - 3)
                    nc.tensor.matmul(
                        ps[i % 4], lhsT,
                        wsb[:, l * 128:(l + 1) * 128],
                        start=True, stop=True,
                    ).then_inc(mm.h, 1)
                    mm.n += 1
                # two per-layer dummy matmuls: the ACT copy of tile i waits
                # mm >= i+2 (PE drain provably complete); the layer's last
                # tiles need successors that don't depend on later layers.
                for _ in range(2):
                    nc.tensor.matmul(
                        ps_dummy[:, 0:128], wsb[:, 0:128], wsb[:, 0:128],
                        start=True, stop=True,
                    ).then_inc(mm.h, 1)
                    mm.n += 1

        @block.scalar
        def _(sc):
            sc.wait_ge(ld.h, 80)
            for l in range(layers):
                for t in range(NT):
                    i = l * NT + t
                    sc.wait_ge(mm.h, l * (NT + 2) + t + 2)
                    if l >= 1 and t == 0:
                        sc.wait_ge(ls.h, 16 * l)
                    sc.activation(
                        stage[:, t * 128:(t + 1) * 128], ps[i % 4],
                        AF.Copy, bias=0.0, scale=dnode[:, t:t + 1],
                    ).then_inc(ynS.h, 1)
                    ynS.n += 1
                if not do_stats:
                    continue
                if l < 3:
                    sc.wait_ge(sq.h, l + 1)
                    sc.activation(t1v[:], t0v[:], AF.Sqrt).then_inc(fa.h, 1)
                    fa.n += 1
                    sc.wait_ge(fa.h, fa.n)
                    # readback after fence: t1v committed before sqr fires
                    sc.activation(t2av[:], t1v[:], AF.Copy).then_inc(sqr.h, 1)
                    sqr.n += 1
                    if debug_dump and l == layers - 1:
                        continue
                    sc.wait_ge(kb.h, l + 1)
                    sc.activation(acc[:], acc[:], AF.Relu,
                                  bias=bvec[:], scale=kvec[:],
                                  ).then_inc(bn.h, 1)
                else:
                    sc.wait_ge(kb.h, l + 1)
                    sc.activation(acc[:], acc[:], AF.Identity,
                                  bias=gbv[:, 6:7], scale=1.0,
                                  ).then_inc(bn.h, 1)
                bn.n += 1

        @block.vector
        def _(ve):
            ve.wait_ge(ld.h, 80)
            cidx = 0
            for l in range(layers):
                for (ta, tb, colsA, colsB, ic0, icA) in chunk_meta:
                    if not do_gather:
                        break
                    ve.wait_ge(gd.h, 32 * (cidx + 1))
                    g = G[cidx % 2]
                    offA = 0
                    offB = int(sum(128 * SA[t] for t in range(ta, tb)))
                    for t in range(ta, tb):
                        wA = 128 * int(SA[t])
                        wB = 128 * int(SB[t])
                        ve.tensor_reduce(
                            out=accA[:],
                            in_=g[:, offA:offA + wA].rearrange(
                                "p (n s) -> p n s", n=128),
                            axis=mybir.AxisListType.X, op=AL.add)
                        ve.tensor_reduce(
                            out=accB[:],
                            in_=g[:, offB:offB + wB].rearrange(
                                "p (n s) -> p n s", n=128),
                            axis=mybir.AxisListType.X, op=AL.add)
                        offA += wA
                        offB += wB
                        tt = ve.tensor_tensor(
                            out=acc[:, t * 128:(t + 1) * 128],
                            in0=accA[:],
                            in1=accB[:], op=AL.add)
                    tt.then_inc(gq.h, 1)
                    gq.n += 1
                    cidx += 1
                # dinv_dst scale (in-place, bf16 second operand)
                dmul = ve.tensor_tensor(out=acc[:], in0=acc[:], in1=drep[:],
                                        op=AL.mult)
                if do_stats and l < 3:
                    # Small (4-8B/partition) DVE writes commit lazily: a
                    # dependent read in the very next op sees stale data.
                    # Fence each small write with a self-semaphore wait.
                    def ff(inst):
                        inst.then_inc(fv.h, 1)
                        fv.n += 1
                        ve.wait_ge(fv.h, fv.n)
                    # stats over real slots
                    for j in range(13):
                        a = j * 512
                        b = min(REAL, a + 512)
                        ins_ = ve.bn_stats(stats6[:, j * 6:(j + 1) * 6],
                                           acc[:, a:b])
                    ff(ins_)
                    ff(ve.bn_aggr(mv[:, 0:2], stats6[:]))
                    # xch_s = [mean, mean^2 + var] = [Ex, Ex2]
                    if l > 0:
                        ve.wait_ge(sls.h, 16 * l)  # prev stats send done
                    ve.tensor_copy(xch_s[:, 0:1], mv[:, 0:1])
                    ff(ve.tensor_tensor(out=t2v[:], in0=mv[:, 0:1],
                                        in1=mv[:, 0:1], op=AL.mult))
                    ff(ve.tensor_tensor(out=xch_s[:, 1:2], in0=mv[:, 1:2],
                                        in1=t2v[:], op=AL.add))
                    # readback signals xch_s committed
                    ve.tensor_copy(t2v[:], xch_s[:, 0:1]).then_inc(st.h, 1)
                    st.n += 1
                    ve.wait_ge(srs.h, 16 * (l + 1))
                    # global stats: average 8 partials
                    ff(ve.tensor_reduce(
                        out=s2v[:],
                        in_=xch_r[:].rearrange("p (c k) -> p k c", c=8),
                        axis=mybir.AxisListType.X, op=AL.add))
                    ff(ve.tensor_scalar(out=s2v[:], in0=s2v[:],
                                        scalar1=0.125, scalar2=None,
                                        op0=AL.mult))
                    # var = Ex2m - gmean^2 + eps ; t0 = 1/var
                    ff(ve.tensor_tensor(out=t2v[:], in0=s2v[:, 0:1],
                                        in1=s2v[:, 0:1], op=AL.mult))
                    ff(ve.tensor_tensor(out=t0v[:], in0=s2v[:, 1:2],
                                        in1=t2v[:], op=AL.subtract))
                    ff(ve.tensor_scalar(out=t0v[:], in0=t0v[:],
                                        scalar1=BN_EPS, scalar2=None,
                                        op0=AL.add))
                    ff(ve.reciprocal(t0v[:], t0v[:]))
                    ve.tensor_copy(t2v[:], t0v[:]).then_inc(sq.h, 1)
                    sq.n += 1
                    # ACT computes t1 = sqrt(t0) = rstd
                    ve.wait_ge(sqr.h, l + 1)
                    ff(ve.tensor_tensor(out=kvec[:],
                                        in0=gbv[:, 2 * l:2 * l + 1],
                                        in1=t1v[:], op=AL.mult))
                    ff(ve.tensor_tensor(out=t2v[:], in0=s2v[:, 0:1],
                                        in1=kvec[:], op=AL.mult))
                    ff(ve.tensor_tensor(out=bvec[:],
                                        in0=gbv[:, 2 * l + 1:2 * l + 2],
                                        in1=t2v[:], op=AL.subtract))
                    ve.tensor_copy(t2v[:], bvec[:]).then_inc(kb.h, 1)
                else:
                    dmul.then_inc(kb.h, 1)
                kb.n += 1

        @block.gpsimd
        def _(gp):
            gp.wait_ge(ld.h, 80)
            gp.memset(yn[:, 0:128], 0)
            gp.memset(yn[:, B_VIEW_RANK * 128 + 32768 - 128:
                          B_VIEW_RANK * 128 + 32768], 0)
            cidx = 0
            for l in range(layers):
                gp.wait_ge(ynS.h, NT * (l + 1))
                if l > 0:
                    gp.wait_ge(dn.h, 16 * l)
                ynoff = gp.partition_id() * SHARD + 128
                gp.remote_dma_broadcast(
                    out_ap=yn[:, bass.ds(ynoff, SHARD)],
                    in_ap=stage[:],
                    remote_sem=rs.h, local_sem=ls.h,
                    rdests=[(0, k) for k in range(NCORES)],
                ).then_inc(psm.h, 1)
                psm.n += 1
                gp.wait_ge(psm.h, psm.n)
                gp.trigger_dma(count=1)
                gp.wait_ge(rs.h, 16 * (l + 1))
                for (ta, tb, colsA, colsB, ic0, icA) in chunk_meta:
                    if not do_gather:
                        break
                    if cidx >= 2:
                        gp.wait_ge(gq.h, cidx - 1)
                    g = G[cidx % 2]
                    gp.dma_gather(
                        out_ap=g[:, 0:colsA].rearrange(
                            "p (o n) -> p o n", o=1),
                        in_ap=yn[:, 0:32768],
                        idxs_ap=idx_sb[:, ic0:ic0 + colsA // 16],
                        num_idxs=colsA, num_idxs_reg=colsA,
                        elem_size=128, transpose=True,
                        sbuf_tokens_per_rank=128,
                        sbuf_free_dim_per_rank=256,
                        single_packet=False,
                    ).then_inc(gd.h, 16)
                    gd.n += 16
                    gp.dma_gather(
                        out_ap=g[:, colsA:colsA + colsB].rearrange(
                            "p (o n) -> p o n", o=1),
                        in_ap=yn[:, B_VIEW_RANK * 128:B_VIEW_RANK * 128 + 32768],
                        idxs_ap=idx_sb[:, icA:icA + colsB // 16],
                        num_idxs=colsB, num_idxs_reg=colsB,
                        elem_size=128, transpose=True,
                        sbuf_tokens_per_rank=128,
                        sbuf_free_dim_per_rank=256,
                        single_packet=False,
                    ).then_inc(gd.h, 16)
                    gd.n += 16
                    cidx += 1
                gp.wait_ge(gd.h, gd.n)
                gp.remote_sem_update_broadcast(
                    remote_sem=dn.h, local_sem=dl.h,
                    rdests=[(0, k) for k in range(NCORES)],
                ).then_inc(psm.h, 1)
                psm.n += 1
                gp.wait_ge(psm.h, psm.n)
                gp.trigger_dma(count=1)
                if do_stats and l < 3:
                    gp.wait_ge(st.h, l + 1)
                    xoff = gp.partition_id() * 2
                    gp.remote_dma_broadcast(
                        out_ap=xch_r[:, bass.ds(xoff, 2)],
                        in_ap=xch_s[:],
                        remote_sem=srs.h, local_sem=sls.h,
                        rdests=[(0, k) for k in range(NCORES)],
                    ).then_inc(psm.h, 1)
                    psm.n += 1
                    gp.wait_ge(psm.h, psm.n)
                    gp.trigger_dma(count=1)

    nc.compile()
    return nc


def make_core_inputs(pp, x, Ws, gb):
    """per-core in_maps for run_bass_kernel_spmd / run_bass_via_pjrt"""
    nos = pp["node_of_slot"]
    dinv_slot = pp["dinv_slot"]
    wall = np.zeros((128, 512), np.float32)
    wall[:, 0:128] = Ws[0]
    wall[:, 128:256] = Ws[1]
    wall[:, 256:384] = Ws[2]
    wall[:, 384:448] = Ws[3][:, :64] if Ws[3].shape[1] == 64 else Ws[3][:, :]
    in_maps = []
    for c in range(NCORES):
        slots = c * SHARD + np.arange(SHARD)
        nodes = nos[slots]
        msk = nodes >= 0
        xT = np.zeros((128, SHARD), np.float32)
        xT[:, msk] = x[nodes[msk]].T
        drep = np.broadcast_to(
            dinv_slot[slots].astype(bf16), (128, SHARD)).copy()
        dnode = dinv_slot[slots].reshape(NT, 128).T.copy().astype(np.float32)
        in_maps.append(dict(xT=xT, idx=pp["idx_dram"][c].copy(),
                            drep=drep, dnode=dnode, wall=wall.copy(),
                            gb=gb.copy()))
    return in_maps


def make_gb(g1, be1, g2, be2, g3, be3, b4):
    gb = np.zeros((128, 8), np.float32)
    for i, v in enumerate([g1, be1, g2, be2, g3, be3]):
        gb[:, i] = v
    gb[:64, 6] = b4
    return gb


def assemble_output(pp, results):
    nos = pp["node_of_slot"]
    full = np.zeros((N, OUT), np.float32)
    for c in range(NCORES):
        slots = c * SHARD + np.arange(SHARD)
        nodes = nos[slots]
        msk = nodes >= 0
        full[nodes[msk]] = results[c]["out"][:OUT, msk].T
    return full


# ---------------------------------------------------------------------------
# public entry point
# ---------------------------------------------------------------------------
_CACHE = {}


def _get_program(edge_index):
    key = hash(edge_index.tobytes())
    if key not in _CACHE:
        pp = preprocess(edge_index)
        nc = build_program(pp)
        _CACHE[key] = (pp, nc)
    return _CACHE[key]


def kernel(**inputs):
    """Full GCN encoder on 8 TRN2 NeuronCores.

    Takes the full (unsharded) inputs of reference.setup_inputs(), returns
    the full [50000, 64] float32 output.
    """
    from concourse import bass2jax

    inputs = {k: np.asarray(v) for k, v in inputs.items()}
    edge_index = inputs["edge_index"].astype(np.int32)
    pp, nc = _get_program(edge_index)
    Ws = [inputs["W1"], inputs["W2"], inputs["W3"], inputs["W4"]]
    gb = make_gb(inputs["g1"], inputs["be1"], inputs["g2"], inputs["be2"],
                 inputs["g3"], inputs["be3"], inputs["b4"])
    # bias handling: b1..b3 cancel inside batch-norm (per-feature constant
    # shifts drop out of x - mean); b4 is applied on-device via gb col 6.
    in_maps = make_core_inputs(pp, inputs["x"].astype(np.float32), Ws, gb)
    results = bass2jax.run_bass_via_pjrt(nc, in_maps, n_cores=NCORES)
    return assemble_output(pp, results)

